# revision 1
# baseline (speedup 1.0000x reference)
"""Trainium2 Bass kernel: GSpade node embedding.

Computation (see reference):
  - bidirectional tanh-RNN (hidden 512/dir) over T=32768 tokens grouped into
    N=2048 contiguous ragged segments (sorted group ids in `masks`)
  - mean-pool hidden states per segment -> pooled [N, 1024]
  - out = [x @ Wx.T + bx | pooled]  -> [N, 2048]

Sharding (8 NeuronCores, SPMD single program):
  - cores 0-3: forward RNN, cores 4-7: backward RNN.  Segments are sorted by
    length (desc) and striped: scan-stripe c (= core % 4) owns segments with
    sorted rank 4i+c, i.e. 512 "lanes" per core.  Forward core c and backward
    core c+4 own the same segments, so pooled = [pooled_f | pooled_b] splits
    column-wise with no cross-core traffic.
  - each core also computes 256 rows of the x-projection.

Per-core scan layout: hidden state h is kept transposed [feature(4x128p), lane]
so the PE contraction dim (features) stays on partitions across steps - no
per-step transpose.  Lanes are end-aligned to a shared schedule L_i (max of the
4 stripes at rank i): a lane's segment is zero-PREFIXED, so h stays exactly 0
(tanh(0 + W@0) = 0, bias is masked) until its first real token, and every lane
retires exactly at step L_i.  Active lane count N(t) = #{L_i > t} shrinks with
t, and the sorted layout makes the active set a prefix -> pure slicing, no
masking, ~0.1% padded work.

Per step t (N = N(t) lanes), accumulated in PSUM [128, 4x512] (4 banks):
  psum[jc] = W_ih.T[:,jc] @ x_t  (input proj, raw tokens, start=True)
           + b[jc] (x) mask_t    (rank-1 matmul; mask kills zero-prefix lanes)
           + sum_kc W_hh.T[kc,jc] @ h[kc]   (16 matmuls)
  h' = tanh(psum)                (1-2 fused ACT ops over the 4 banks)
  acc += h'                      (DVE, mean-pool accumulator, sliced to N(t))
"""

import ml_dtypes
import numpy as np

import concourse.bacc as bacc
import concourse.mybir as mybir
from concourse.tile import TileContext
from concourse.bass_utils import run_bass_kernel_spmd

FP32 = mybir.dt.float32
F32R = mybir.dt.float32r
BF16 = mybir.dt.bfloat16
Tanh = mybir.ActivationFunctionType.Tanh

N_GROUPS = 2048
D_SEQ = 128
H = 512           # hidden per direction
HC = 4            # hidden chunks of 128
D_PROJ = 1024
N_CORES = 8
LANES = 512       # segments per scan core
XROWS = N_GROUPS // N_CORES  # x-projection rows per core

_program_cache: dict = {}


def _dma_chunks(nt, target=2048):
    """Group steps into DMA chunks of ~target columns. Returns [(t0, t1), ...]."""
    chunks = []
    t0 = 0
    cols = 0
    for t, n in enumerate(nt):
        if cols > 0 and cols + n > target:
            chunks.append((t0, t))
            t0, cols = t, 0
        cols += n
    chunks.append((t0, len(nt)))
    return chunks


def _build_program(nt, nt_true):
    """Build + compile the SPMD program. nt = even-padded N(t) (fp32r matmul
    needs even free dims); nt_true = real active-lane counts (acc slicing)."""
    nt = list(nt)
    nt_true = list(nt_true)
    steps = len(nt)
    off = np.concatenate([[0], np.cumsum(nt)]).astype(int)
    S = int(off[-1])

    nc = bacc.Bacc("TRN2", target_bir_lowering=False, debug=False,
                   num_devices=N_CORES)

    xtok_d = nc.dram_tensor("xtok", [128, S], BF16, kind="ExternalInput")
    mrow_d = nc.dram_tensor("mrow", [1, S], F32R, kind="ExternalInput")
    whhT_d = nc.dram_tensor("whhT", [H, H], BF16, kind="ExternalInput")
    wihT_d = nc.dram_tensor("wihT", [D_SEQ, H], BF16, kind="ExternalInput")
    brow_d = nc.dram_tensor("brow", [1, H], F32R, kind="ExternalInput")
    invl_d = nc.dram_tensor("invl", [128, LANES], FP32, kind="ExternalInput")
    xT_d = nc.dram_tensor("xT", [H, XROWS], F32R, kind="ExternalInput")
    wxT_d = nc.dram_tensor("wxT", [H, D_PROJ], F32R, kind="ExternalInput")
    bxrow_d = nc.dram_tensor("bxrow", [1, D_PROJ], F32R, kind="ExternalInput")
    ones_d = nc.dram_tensor("ones", [1, 128], F32R, kind="ExternalInput")

    xp_d = nc.dram_tensor("xp", [XROWS, D_PROJ], FP32, kind="ExternalOutput")
    pooledT_d = nc.dram_tensor("pooledT", [H, LANES], FP32, kind="ExternalOutput")

    with TileContext(nc) as tc:
        with (
            tc.tile_pool(name="sb", bufs=1) as sb,
            tc.tile_pool(name="ps", bufs=2, space="PSUM") as psp,
        ):
            # ---- persistent SBUF tiles + loads ----
            wx_sb = [sb.tile([128, D_PROJ], F32R, tag=f"wx{k}", name=f"wx{k}") for k in range(4)]
            xT_sb = [sb.tile([128, XROWS], F32R, tag=f"xT{k}", name=f"xT{k}") for k in range(4)]
            ones_sb = sb.tile([1, 128], F32R, tag="ones", name="ones")
            bx_sb = sb.tile([1, D_PROJ], F32R, tag="bx", name="bx")
            for k in range(4):
                nc.sync.dma_start(out=wx_sb[k][:, :], in_=wxT_d[k * 128:(k + 1) * 128, :])
                nc.sync.dma_start(out=xT_sb[k][:, :], in_=xT_d[k * 128:(k + 1) * 128, :])
            nc.sync.dma_start(out=ones_sb[:, :], in_=ones_d[:, :])
            nc.sync.dma_start(out=bx_sb[:, :], in_=bxrow_d[:, :])

            wh_sb = [sb.tile([128, H], BF16, tag=f"wh{k}", name=f"wh{k}") for k in range(HC)]
            wih_sb = sb.tile([128, H], BF16, tag="wih", name="wih")
            brow_sb = sb.tile([1, H], F32R, tag="brow", name="brow")
            invl_sb = sb.tile([128, LANES], FP32, tag="invl", name="invl")
            mrow_sb = sb.tile([1, S], F32R, tag="mrow", name="mrow")
            xtok_sb = sb.tile([128, S], BF16, tag="xtok", name="xtok")
            for k in range(HC):
                nc.sync.dma_start(out=wh_sb[k][:, :], in_=whhT_d[k * 128:(k + 1) * 128, :])
            nc.sync.dma_start(out=wih_sb[:, :], in_=wihT_d[:, :])
            nc.sync.dma_start(out=brow_sb[:, :], in_=brow_d[:, :])
            nc.sync.dma_start(out=invl_sb[:, :], in_=invl_d[:, :])
            nc.sync.dma_start(out=mrow_sb[:, :], in_=mrow_d[:, :])
            for (t0, t1) in _dma_chunks(nt):
                a, b = int(off[t0]), int(off[t1])
                nc.sync.dma_start(out=xtok_sb[:, a:b], in_=xtok_d[:, a:b])

            # scan state
            h_sb = [sb.tile([128, HC * H], BF16, tag=f"h{p}", name=f"h{p}") for p in range(2)]
            acc_sb = sb.tile([128, HC * H], FP32, tag="acc", name="acc")

            # acc = 0, routed through ACT tanh so the tanh table set loads
            # up-front (overlapping the x-proj phase) instead of stalling
            # the first scan step.
            nc.vector.memset(acc_sb[:, :], 0.0)
            nc.scalar.activation(acc_sb[:, :], acc_sb[:, :], Tanh)

            # ---- phase A: x projection (also the PE/HAM warm-up) ----
            xp_sb = [sb.tile([128, D_PROJ], FP32, tag=f"xp{b}", name=f"xpsb{b}") for b in range(2)]
            for bc in range(2):
                ps = psp.tile([128, HC * H], FP32, tag="ps", name="ps")
                for jh in range(2):
                    o = ps[:, jh * H:jh * H + H]
                    nc.tensor.matmul(o, ones_sb[0:1, :], bx_sb[0:1, jh * H:(jh + 1) * H],
                                     start=True, stop=False)
                    for kc in range(4):
                        nc.tensor.matmul(o, xT_sb[kc][:, bc * 128:(bc + 1) * 128],
                                         wx_sb[kc][:, jh * H:(jh + 1) * H],
                                         start=False, stop=(kc == 3))
                    nc.vector.tensor_copy(xp_sb[bc][:, jh * H:(jh + 1) * H], o)
                nc.sync.dma_start(out=xp_d[bc * 128:(bc + 1) * 128, :], in_=xp_sb[bc][:, :])

            # ---- scan ----
            for t in range(steps):
                n = nt[t]
                na = nt_true[t]
                a = int(off[t])
                hr = h_sb[(t + 1) % 2]   # state produced by step t-1
                hw = h_sb[t % 2]         # state produced by this step
                xcur = xtok_sb[:, a:a + n]
                mcur = mrow_sb[0:1, a:a + n]
                ps = psp.tile([128, HC * H], FP32, tag="ps", name="ps")

                # input projection + masked bias (independent of h)
                for jc in range(HC):
                    o = ps[:, jc * H:jc * H + n]
                    nc.tensor.matmul(o, wih_sb[:, jc * 128:(jc + 1) * 128], xcur,
                                     start=True, stop=False)
                for jc in range(HC):
                    o = ps[:, jc * H:jc * H + n]
                    nc.tensor.matmul(o, brow_sb[0:1, jc * 128:(jc + 1) * 128], mcur,
                                     start=False, stop=(t == 0))
                ps3 = ps.rearrange("p (c n) -> p c n", c=HC)
                hw3 = hw.rearrange("p (c n) -> p c n", c=HC)
                acc3 = acc_sb.rearrange("p (c n) -> p c n", c=HC)

                if t > 0:
                    # hidden recurrence, k-chunk outer so each j-bank finishes late
                    # but the 8 h-independent matmuls above cover the tanh latency
                    for kc in range(HC):
                        hk = hr[:, kc * H:kc * H + n]
                        for jc in range(HC):
                            nc.tensor.matmul(ps[:, jc * H:jc * H + n],
                                             wh_sb[kc][:, jc * 128:(jc + 1) * 128], hk,
                                             start=False, stop=(kc == HC - 1))
                            if kc == HC - 1 and n >= 256 and jc == 1:
                                nc.scalar.activation(hw3[:, 0:2, 0:n], ps3[:, 0:2, 0:n], Tanh)
                    if n >= 256:
                        nc.scalar.activation(hw3[:, 2:4, 0:n], ps3[:, 2:4, 0:n], Tanh)
                    else:
                        nc.scalar.activation(hw3[:, 0:4, 0:n], ps3[:, 0:4, 0:n], Tanh)
                else:
                    nc.scalar.activation(hw3[:, 0:4, 0:n], ps3[:, 0:4, 0:n], Tanh)

                if n >= 256:
                    nc.vector.tensor_add(acc3[:, 0:2, 0:na], acc3[:, 0:2, 0:na], hw3[:, 0:2, 0:na])
                    nc.vector.tensor_add(acc3[:, 2:4, 0:na], acc3[:, 2:4, 0:na], hw3[:, 2:4, 0:na])
                else:
                    nc.vector.tensor_add(acc3[:, 0:4, 0:na], acc3[:, 0:4, 0:na], hw3[:, 0:4, 0:na])

            # ---- finalize: pooledT[jc] = acc[jc] * (1/len) ----
            for jc in range(HC):
                po = sb.tile([128, LANES], FP32, tag=f"po{jc}", name=f"po{jc}")
                nc.vector.tensor_mul(po[:, :], acc_sb[:, jc * H:(jc + 1) * H], invl_sb[:, :])
                nc.sync.dma_start(out=pooledT_d[jc * 128:(jc + 1) * 128, :], in_=po[:, :])

    nc.compile()
    return nc


def _get_program(nt, nt_true):
    key = (tuple(nt), tuple(nt_true))
    if key not in _program_cache:
        _program_cache[key] = _build_program(nt, nt_true)
    return _program_cache[key]


def _prepare(x, seqs, masks, W_ih_f, W_hh_f, b_f, W_ih_b, W_hh_b, b_b, Wx, bx):
    x = np.asarray(x, np.float32)
    seqs = np.asarray(seqs, np.float32)
    masks = np.asarray(masks).astype(np.int64)

    T = seqs.shape[0]

    # ---- segment geometry (host) ----
    lens = np.bincount(masks, minlength=N_GROUPS).astype(np.int64)
    starts_all = np.concatenate([[0], np.cumsum(lens)[:-1]])
    order = np.argsort(-lens, kind="stable")          # groups sorted by len desc
    sl = lens[order]
    L = sl[0::4].astype(np.int64)                     # shared lane schedule (512)
    steps = int(L[0])
    nt_true = [int((L > t).sum()) for t in range(steps)]
    nt = [(n + 1) // 2 * 2 for n in nt_true]          # fp32r: even matmul widths
    off_true = np.concatenate([[0], np.cumsum(nt_true)]).astype(int)
    off = np.concatenate([[0], np.cumsum(nt)]).astype(int)
    S = int(off[-1])

    def pad_stream(flat2d):
        # [rows, S_true] -> [rows, S] inserting one zero col per odd step
        out = np.zeros((flat2d.shape[0], S), flat2d.dtype)
        for t in range(steps):
            out[:, off[t]:off[t] + nt_true[t]] = flat2d[:, off_true[t]:off_true[t] + nt_true[t]]
        return out

    # active/real masks per (step, lane); active lanes are a prefix each step
    t_grid = np.arange(steps)[:, None]
    active = t_grid < L[None, :]                      # [steps, LANES]

    seqs_pad = np.vstack([np.zeros((1, D_SEQ), np.float32), seqs])

    gid = [order[c::4] for c in range(4)]
    in_maps = []
    per_stripe = {}
    for c in range(4):
        lens_c = lens[gid[c]]
        starts_c = starts_all[gid[c]]
        pre = (L - lens_c)[None, :]                   # zero-prefix length
        real = active & (t_grid >= pre)
        pos = t_grid - pre
        idx_f = np.where(real, starts_c[None, :] + pos, -1)
        idx_b = np.where(real, starts_c[None, :] + lens_c[None, :] - 1 - pos, -1)
        real_flat = real[active]
        xtok_f = pad_stream(np.ascontiguousarray(seqs_pad[idx_f[active] + 1].T))
        xtok_b = pad_stream(np.ascontiguousarray(seqs_pad[idx_b[active] + 1].T))
        mrow = pad_stream(real_flat.astype(np.float32)[None, :])
        invl = np.ascontiguousarray(
            np.broadcast_to((1.0 / lens_c).astype(np.float32)[None, :], (128, LANES)))
        per_stripe[c] = (xtok_f, xtok_b, mrow, invl)

    ones = np.ones((1, 128), np.float32)
    wxT = np.ascontiguousarray(np.asarray(Wx, np.float32).T)
    bxr = np.asarray(bx, np.float32)[None, :]
    for core in range(N_CORES):
        c = core % 4
        fwd = core < 4
        xtok_f, xtok_b, mrow, invl = per_stripe[c]
        W_ih = W_ih_f if fwd else W_ih_b
        W_hh = W_hh_f if fwd else W_hh_b
        b = b_f if fwd else b_b
        in_maps.append({
            "xtok": (xtok_f if fwd else xtok_b).astype(ml_dtypes.bfloat16),
            "mrow": mrow,
            "whhT": np.ascontiguousarray(np.asarray(W_hh, np.float32).T).astype(ml_dtypes.bfloat16),
            "wihT": np.ascontiguousarray(np.asarray(W_ih, np.float32).T).astype(ml_dtypes.bfloat16),
            "brow": np.asarray(b, np.float32)[None, :],
            "invl": invl,
            "xT": np.ascontiguousarray(
                x[core * XROWS:(core + 1) * XROWS, :].T),
            "wxT": wxT,
            "bxrow": bxr,
            "ones": ones,
        })

    return (nt, nt_true), in_maps, gid


def _assemble(res, gid):
    out = np.empty((N_GROUPS, 2 * D_PROJ), np.float32)
    for core in range(N_CORES):
        out[core * XROWS:(core + 1) * XROWS, :D_PROJ] = res[core]["xp"]
    for c in range(4):
        out[gid[c], D_PROJ:D_PROJ + H] = res[c]["pooledT"].T
        out[gid[c], D_PROJ + H:] = res[c + 4]["pooledT"].T
    return out


def kernel(**inputs):
    (nt, nt_true), in_maps, gid = _prepare(**inputs)
    nc = _get_program(nt, nt_true)
    res = run_bass_kernel_spmd(nc, in_maps, list(range(N_CORES))).results
    return _assemble(res, gid)



# revision 4
# speedup vs baseline: 1.0037x; 1.0037x over previous
"""Trainium2 Bass kernel: GSpade node embedding — fp8 DR, dual interleaved scans.

Computation (see reference):
  bidirectional tanh-RNN (512/dir) over 32768 tokens in 2048 sorted ragged
  segments; mean-pool per segment; concat with x @ Wx.T + bx -> [2048, 2048].

Sharding (8 NeuronCores, SPMD): cores 0-3 forward, 4-7 backward.  Segments
sorted by length desc and striped 8 ways; scan core c runs TWO interleaved
sub-scans: alpha = ranks 8i+c, beta = ranks 8i+4+c (256 lanes each).  The two
sub-scans alternate on every engine, so the serial tanh->matmul->tanh latency
of one is hidden under the other's activation work.  Lanes are end-aligned to
shared schedules L_a[i]=len(rank 8i), L_b[i]=len(rank 8i+4); zero-prefixed
lanes keep h==0 until their first token (DR-pair masked bias), so active
lanes are a shrinking prefix -> pure slicing.

Per sub-step (n active lanes), all scan math in fp8e4 DoubleRow (weights
x256, tanh descales):
  - input proj + masked bias: 4 DR matmuls (pair = tokens | mask strip)
  - recurrence: 2 DR passes x 4 out chunks over h~(t-1)
  - tanh: ONE ACT inst [128,4,n] psum->fp8 h~, scale=1/256
  - h~ ring of 4 slots so pooled-accumulate reads never gate the next tanh
Pooling: identity-pair DR matmuls form h(t)+h(t+1) into a rotating psum
tile; DVE (chunks 0-1) and Pool (chunks 2-3) drain it into an SBUF fp32
accumulator; retired lanes are finalized (x 1/len) and DMA'd out in stages.
PSUM: one pool of [128,2048] tiles (4 banks) x bufs=2 — the alpha/beta
alternation itself provides double buffering.
x-projection (bf16, transposed, bias via DVE) is sliced between early scan
steps once the PE p-state is warm.
"""

import ml_dtypes
import numpy as np

import concourse.bacc as bacc
import concourse.mybir as mybir
from concourse.tile import TileContext
from concourse.bass_utils import run_bass_kernel_spmd

FP32 = mybir.dt.float32
BF16 = mybir.dt.bfloat16
FP8 = mybir.dt.float8e4
DR = mybir.MatmulPerfMode.DoubleRow
Tanh = mybir.ActivationFunctionType.Tanh
E4 = ml_dtypes.float8_e4m3

N_GROUPS = 2048
D_SEQ = 128
H = 512
HC = 4
D_PROJ = 1024
N_CORES = 8
LANES = 256       # per sub-scan
XROWS = N_GROUPS // N_CORES
SCALE = 256.0

_program_cache: dict = {}


def _dma_chunks(nt, first=1, target=1536):
    chunks = []
    t0 = 0
    cols = 0
    for t, n in enumerate(nt):
        if t == first or (cols > 0 and cols + n > target):
            chunks.append((t0, t))
            t0, cols = t, 0
        cols += n
    chunks.append((t0, len(nt)))
    return chunks


def _build_program(nta, ntb):
    nta, ntb = list(nta), list(ntb)
    sa, sb_ = len(nta), len(ntb)
    offa = np.concatenate([[0], np.cumsum(nta)]).astype(int)
    offb = np.concatenate([[0], np.cumsum(ntb)]).astype(int)
    Sa, Sb = int(offa[-1]), int(offb[-1])
    Spa = (Sa + 15) // 16 * 16
    Spb = (Sb + 15) // 16 * 16

    nc = bacc.Bacc("TRN2", target_bir_lowering=False, debug=False,
                   num_devices=N_CORES)

    xta_d = nc.dram_tensor("xta", [128, 2 * Spa], FP8, kind="ExternalInput")
    xtb_d = nc.dram_tensor("xtb", [128, 2 * Spb], FP8, kind="ExternalInput")
    wih_d = nc.dram_tensor("wih", [128, 2 * H + 256], FP8, kind="ExternalInput")
    whp_d = nc.dram_tensor("whp", [128, 4 * H], FP8, kind="ExternalInput")
    invbx_d = nc.dram_tensor("invbx", [128, 2 * LANES + 8], FP32, kind="ExternalInput")
    xT_d = nc.dram_tensor("xT", [128, 4 * XROWS], BF16, kind="ExternalInput")
    wxT_d = nc.dram_tensor("wxT", [128, 4 * D_PROJ], BF16, kind="ExternalInput")

    xpT_d = nc.dram_tensor("xpT", [D_PROJ, XROWS], BF16, kind="ExternalOutput")
    # pooled: alpha lanes in cols [0,256), beta in [256,512)
    pooledT_d = nc.dram_tensor("pooledT", [H, 2 * LANES], FP32, kind="ExternalOutput")

    with TileContext(nc) as tc:
        with (
            tc.tile_pool(name="sb", bufs=1) as sb,
            tc.tile_pool(name="ps", bufs=2, space="PSUM") as psp,
        ):
            # ---- SBUF tiles + merged, head-ordered DMA ----
            xta_sb = sb.tile([128, 2 * Spa], FP8, tag="xta", name="xta")
            xtb_sb = sb.tile([128, 2 * Spb], FP8, tag="xtb", name="xtb")
            x3a = xta_sb.rearrange("p (i s) -> p i s", i=2)
            x3b = xtb_sb.rearrange("p (i s) -> p i s", i=2)
            d3a = xta_d.rearrange("p (i s) -> p i s", i=2)
            d3b = xtb_d.rearrange("p (i s) -> p i s", i=2)
            wih_sb = sb.tile([128, 2 * H + 256], FP8, tag="wih", name="wih")
            whp_sb = sb.tile([128, 4 * H], FP8, tag="whp", name="whp")
            invbx_sb = sb.tile([128, 2 * LANES + 8], FP32, tag="invbx", name="invbx")
            wx_sb = sb.tile([128, 4 * D_PROJ], BF16, tag="wx", name="wx")
            xT_sb = sb.tile([128, 4 * XROWS], BF16, tag="xT", name="xT")

            cha = _dma_chunks(nta)
            chb = _dma_chunks(ntb)

            def dma_tok(which, idx):
                if which == 0 and idx < len(cha):
                    t0, t1 = cha[idx]
                    a, b = int(offa[t0]), int(offa[t1])
                    nc.sync.dma_start(out=x3a[:, :, a:b], in_=d3a[:, :, a:b])
                if which == 1 and idx < len(chb):
                    t0, t1 = chb[idx]
                    a, b = int(offb[t0]), int(offb[t1])
                    nc.sync.dma_start(out=x3b[:, :, a:b], in_=d3b[:, :, a:b])

            nc.sync.dma_start(out=wih_sb[:, :], in_=wih_d[:, :])
            dma_tok(0, 0)
            dma_tok(1, 0)
            nc.sync.dma_start(out=whp_sb[:, :], in_=whp_d[:, :])
            dma_tok(0, 1)
            dma_tok(1, 1)
            dma_tok(0, 2)
            dma_tok(1, 2)
            nc.sync.dma_start(out=invbx_sb[:, :], in_=invbx_d[:, :])
            for i in range(3, max(len(cha), len(chb))):
                dma_tok(0, i)
                dma_tok(1, i)
            nc.sync.dma_start(out=wx_sb[:, :], in_=wxT_d[:, :])
            nc.sync.dma_start(out=xT_sb[:, :], in_=xT_d[:, :])

            # h~ rings: [p, slot(4), chunk(4), lane(256)] per sub-scan
            h2a = sb.tile([128, 4 * HC * LANES], FP8, tag="h2a", name="h2a")
            h2b = sb.tile([128, 4 * HC * LANES], FP8, tag="h2b", name="h2b")
            h4a = h2a.rearrange("p (s c j) -> p s c j", s=4, c=HC)
            h4b = h2b.rearrange("p (s c j) -> p s c j", s=4, c=HC)
            ip3 = wih_sb[:, 2 * H:2 * H + 256].rearrange("p (i o) -> p i o", i=2)

            # SBUF pooled accumulators (fp32): [p, chunk(4), lane(256)] x2
            acc_sb = sb.tile([128, 2 * HC * LANES], FP32, tag="acc", name="acc")
            ac4 = acc_sb.rearrange("p (x c j) -> p x c j", x=2, c=HC)
            nc.vector.memset(ac4[:, 0], 0.0)
            nc.gpsimd.memset(ac4[:, 1], 0.0)

            po_sb = sb.tile([128, HC * 2 * LANES], FP32, tag="po", name="po")
            po4 = po_sb.rearrange("p (c j) -> p c j", c=HC)    # j in [0,512)
            pd3 = pooledT_d.rearrange("(c p) j -> p c j", c=HC)

            xpo_sb = sb.tile([128, 8 * XROWS], BF16, tag="xpo", name="xpo")
            xpd3 = xpT_d.rearrange("(g p) j -> p g j", g=8)
            xpo3 = xpo_sb.rearrange("p (g j) -> p g j", g=8)

            # ACT tanh table pre-warm
            warm_sb = sb.tile([128, 2], FP32, tag="warm", name="warm")
            nc.vector.memset(warm_sb[:, :], 0.0)
            nc.scalar.activation(warm_sb[:, :], warm_sb[:, :], Tanh)

            wih3 = wih_sb[:, 0:2 * H].rearrange("p (i o) -> p i o", i=2)
            wh4 = whp_sb.rearrange("p (k i o) -> p k i o", k=2, i=2)

            def substep(t, nt, off, x3, h4):
                n = nt[t]
                a = int(off[t])
                s = t % 4
                r = (t - 1) % 4
                ps = psp.tile([128, HC * 512], FP32, tag="ps", name="ps")
                ps3 = ps.rearrange("p (c j) -> p c j", c=HC)
                for jc in range(HC):
                    nc.tensor.matmul(
                        ps[:, jc * 512:jc * 512 + n],
                        wih3[:, :, jc * 128:(jc + 1) * 128],
                        x3[:, :, a:a + n],
                        start=True, stop=(t == 0), perf_mode=DR)
                if t > 0:
                    for kc2 in range(2):
                        for jc in range(HC):
                            nc.tensor.matmul(
                                ps[:, jc * 512:jc * 512 + n],
                                wh4[:, kc2, :, jc * 128:(jc + 1) * 128],
                                h4[:, r, 2 * kc2:2 * kc2 + 2, 0:n],
                                start=False, stop=(kc2 == 1), perf_mode=DR)
                nc.scalar.activation(h4[:, s, :, 0:n], ps3[:, :, 0:n],
                                     Tanh, scale=1.0 / SCALE)

            def accum(t, nt, h4, x):
                """pooled accumulate: acc += h~(t), chunks 0-2 on DVE,
                chunk 3 on Pool; reads exactly the freshly written width."""
                n = nt[t]
                s = t % 4
                nc.vector.tensor_add(ac4[:, x, 0:3, 0:n],
                                     ac4[:, x, 0:3, 0:n],
                                     h4[:, s, 0:3, 0:n])
                nc.gpsimd.tensor_add(ac4[:, x, 3, 0:n],
                                     ac4[:, x, 3, 0:n],
                                     h4[:, s, 3, 0:n])

            def fin_cols(x, lo, hi):
                """finalize pooled cols [lo,hi) of sub-scan x (acc * 1/len)."""
                g = x * LANES
                for c in range(HC):
                    eng = nc.vector if c < 2 else nc.gpsimd
                    eng.tensor_mul(po4[:, c, g + lo:g + hi],
                                   ac4[:, x, c, lo:hi],
                                   invbx_sb[:, g + lo:g + hi])
                nc.sync.dma_start(out=pd3[:, :, g + lo:g + hi],
                                  in_=po4[:, :, g + lo:g + hi])

            def xproj_group(g):
                xps = psp.tile([128, HC * 512], FP32, tag="ps", name="xps")
                for hh in range(2):
                    pc = 2 * g + hh
                    o = xps[:, hh * 512:hh * 512 + XROWS]
                    for kc in range(4):
                        nc.tensor.matmul(
                            o,
                            wx_sb[:, kc * D_PROJ + pc * 128:kc * D_PROJ + (pc + 1) * 128],
                            xT_sb[:, kc * XROWS:(kc + 1) * XROWS],
                            start=(kc == 0), stop=(kc == 3))
                    nc.vector.tensor_scalar_add(
                        xpo3[:, pc, :], o,
                        invbx_sb[:, 2 * LANES + pc:2 * LANES + pc + 1])
                nc.sync.dma_start(out=xpd3[:, 2 * g:2 * g + 2, :],
                                  in_=xpo3[:, 2 * g:2 * g + 2, :])

            # staged finalize thresholds (per sub-scan)
            fina = {}
            finb = {}
            for fin, nt, hi0 in ((fina, nta, LANES), (finb, ntb, LANES)):
                done = hi0
                for u in range(1, len(nt), 2):
                    nxt = nt[u + 1] if u + 1 < len(nt) else 0
                    if done - nxt >= 40 and done > 40:
                        fin[u] = (nxt, done)
                        done = nxt
                fin["end"] = (0, done)

            # ---- interleaved dual scan ----
            for t in range(sa):
                substep(t, nta, offa, x3a, h4a)
                if t < sb_:
                    substep(t, ntb, offb, x3b, h4b)
                accum(t, nta, h4a, 0)
                if t < sb_:
                    accum(t, ntb, h4b, 1)
                if t in fina:
                    fin_cols(0, *fina[t])
                if t < sb_ and t in finb:
                    fin_cols(1, *finb[t])
            for g in range(4):
                xproj_group(g)
            fin_cols(0, *fina["end"])
            fin_cols(1, *finb["end"])

    nc.compile()
    return nc


def _get_program(nta, ntb=None):
    if ntb is None:
        nta, ntb = nta
    key = (tuple(nta), tuple(ntb))
    if key not in _program_cache:
        _program_cache[key] = _build_program(nta, ntb)
    return _program_cache[key]


def _prepare(x, seqs, masks, W_ih_f, W_hh_f, b_f, W_ih_b, W_hh_b, b_b, Wx, bx):
    x = np.asarray(x, np.float32)
    seqs = np.asarray(seqs, np.float32)
    masks = np.asarray(masks).astype(np.int64)

    lens = np.bincount(masks, minlength=N_GROUPS).astype(np.int64)
    starts_all = np.concatenate([[0], np.cumsum(lens)[:-1]])
    order = np.argsort(-lens, kind="stable")
    sl = lens[order]

    seqs_pad = np.vstack([np.zeros((1, D_SEQ), np.float32), seqs])

    def schedule(base):
        L = sl[base::8].astype(np.int64)              # 256 lanes
        steps = int(L[0])
        nt = [int((L > t).sum()) for t in range(steps)]
        off = np.concatenate([[0], np.cumsum(nt)]).astype(int)
        return L, steps, nt, off

    La, sa, nta, offa = schedule(0)
    Lb, sb_, ntb, offb = schedule(4)
    Spa = (int(offa[-1]) + 15) // 16 * 16
    Spb = (int(offb[-1]) + 15) // 16 * 16

    def streams(base, L, steps, nt, off, Sp, cidx):
        """token+strip stream for stripe (base+cidx) with schedule L."""
        S = int(off[-1])
        t_grid = np.arange(steps)[:, None]
        active = t_grid < L[None, :]
        g = order[base + cidx::8]
        lens_c = lens[g]
        starts_c = starts_all[g]
        pre = (L - lens_c)[None, :]
        real = active & (t_grid >= pre)
        pos = t_grid - pre
        idx_f = np.where(real, starts_c[None, :] + pos, -1)
        idx_b = np.where(real, starts_c[None, :] + lens_c[None, :] - 1 - pos, -1)
        real_flat = real[active].astype(np.float32)
        xf = np.zeros((128, Sp), E4)
        xb = np.zeros((128, Sp), E4)
        xf[:, :S] = seqs_pad[idx_f[active] + 1].T.astype(E4)
        xb[:, :S] = seqs_pad[idx_b[active] + 1].T.astype(E4)
        strip = np.zeros((128, Sp), E4)
        strip[0, :S] = real_flat.astype(E4)
        return (np.hstack([xf, strip]), np.hstack([xb, strip]), g, lens_c)

    ip = np.zeros((128, 2, 128), E4)
    for k in range(128):
        ip[k, 0, k] = 1.0
        ip[k, 1, k] = 1.0
    ip = ip.reshape(128, 256)

    def pack_wih(W_ih, b):
        Wq = (np.asarray(W_ih, np.float32) * SCALE).astype(E4)
        bq = (np.asarray(b, np.float32) * SCALE).astype(E4)
        out = np.zeros((128, 2 * H + 256), E4)
        out[:, 0:H] = Wq.T
        out[0, H:2 * H] = bq
        out[:, 2 * H:] = ip
        return out

    def pack_whh(W_hh):
        Wq = (np.asarray(W_hh, np.float32) * SCALE).astype(E4)
        WqT = Wq.T
        out = np.zeros((128, 4 * H), E4)
        for kc2 in range(2):
            for i in range(2):
                out[:, (kc2 * 2 + i) * H:(kc2 * 2 + i + 1) * H] = \
                    WqT[kc2 * 256 + i * 128: kc2 * 256 + (i + 1) * 128, :]
        return out

    wxT = np.asarray(Wx, np.float32).T.astype(ml_dtypes.bfloat16)
    wx_m = np.zeros((128, 4 * D_PROJ), ml_dtypes.bfloat16)
    for kc in range(4):
        wx_m[:, kc * D_PROJ:(kc + 1) * D_PROJ] = wxT[kc * 128:(kc + 1) * 128, :]
    bxa = np.asarray(bx, np.float32)

    wihp_f = pack_wih(W_ih_f, b_f)
    wihp_b = pack_wih(W_ih_b, b_b)
    whp_f = pack_whh(W_hh_f)
    whp_b = pack_whh(W_hh_b)

    in_maps = []
    gids = []
    for c4 in range(4):
        xa_f, xa_b, ga, lens_a = streams(0, La, sa, nta, offa, Spa, c4)
        xb_f, xb_b, gb, lens_b = streams(4, Lb, sb_, ntb, offb, Spb, c4)
        gids.append((ga, gb))
        invbx = np.zeros((128, 2 * LANES + 8), np.float32)
        invbx[:, :LANES] = (1.0 / lens_a).astype(np.float32)[None, :]
        invbx[:, LANES:2 * LANES] = (1.0 / lens_b).astype(np.float32)[None, :]
        for pc in range(8):
            invbx[:, 2 * LANES + pc] = bxa[pc * 128:(pc + 1) * 128]
        for fwd in (True, False):
            core = c4 if fwd else c4 + 4
            xTc = x[core * XROWS:(core + 1) * XROWS, :].T.astype(ml_dtypes.bfloat16)
            xT_m = np.zeros((128, 4 * XROWS), ml_dtypes.bfloat16)
            for kc in range(4):
                xT_m[:, kc * XROWS:(kc + 1) * XROWS] = xTc[kc * 128:(kc + 1) * 128, :]
            in_maps.append((core, {
                "xta": xa_f if fwd else xa_b,
                "xtb": xb_f if fwd else xb_b,
                "wih": wihp_f if fwd else wihp_b,
                "whp": whp_f if fwd else whp_b,
                "invbx": invbx,
                "xT": xT_m,
                "wxT": wx_m,
            }))
    in_maps.sort(key=lambda kv: kv[0])
    in_maps = [m for _, m in in_maps]

    return ((nta, ntb), (nta, ntb)), in_maps, gids


def _assemble(res, gids):
    out = np.empty((N_GROUPS, 2 * D_PROJ), np.float32)
    for core in range(N_CORES):
        out[core * XROWS:(core + 1) * XROWS, :D_PROJ] = res[core]["xpT"].T.astype(np.float32)
    for c4 in range(4):
        ga, gb = gids[c4]
        pf = res[c4]["pooledT"]
        pb = res[c4 + 4]["pooledT"]
        out[ga, D_PROJ:D_PROJ + H] = pf[:, :LANES].T
        out[gb, D_PROJ:D_PROJ + H] = pf[:, LANES:].T
        out[ga, D_PROJ + H:] = pb[:, :LANES].T
        out[gb, D_PROJ + H:] = pb[:, LANES:].T
    return out


def kernel(**inputs):
    (ntab, _), in_maps, gids = _prepare(**inputs)
    nc = _get_program(ntab)
    res = run_bass_kernel_spmd(nc, in_maps, list(range(N_CORES))).results
    return _assemble(res, gids)
